# revision 12
# baseline (speedup 1.0000x reference)
"""GQA causal self-attention with ALiBi — Trainium2 Bass kernel, 8 NeuronCores.

Sharding: one (batch, kv-head) pair per core (2 batches x 4 kv heads = 8 cores).
Each core computes its 4 query heads' attention over the full sequence and a
partial output projection y_partial = att_heads @ Wo[head_rows]; the host sums
the 4 partials per batch.

Banded attention: the ALiBi slopes for the 4 kv groups are {0.71, 0.5, 0.35,
0.25}, so a key at distance d contributes a factor exp(-s*d) <= exp(-32) ~
1e-14 once d >= 128.  Each 512-query chunk therefore attends only its own 4
key blocks plus one preceding block (min distance 128); measured banding error
vs the dense reference is ~1.6e-6.  All operands are bf16 (inputs cast on
host), accumulation in fp32 PSUM; measured total error ~4e-3 vs the 2e-2 gate.

Device-side math (per core, T=2048, HD=64, G=4 query heads, slope s):
  QKV^T = (x @ [Wq_g*scale, Wk_g, Wv_g])^T          (x^T pre-transposed, bf16)
  S^T[j,i] = q_i . k_j * scale - (s*i + SHIFT)      (shift row via matmul aug row)
  P^T = exp(S^T + s*j)                              (s*j = per-partition ACT bias)
  P^T masked causally on the 128-col diagonal window (gpsimd affine_select)
  osum = [1|0*63|V]^T @ P^T: row 0 = l, rows 64:128 = attT_unnorm
  attT = attT_unnorm * (1/l broadcast via 0/1 selection matmul)
  y = attT^T @ Wo_rows                              (attT is lhsT directly)

The per-query shift -(s*i+SHIFT) cancels exactly in attT_unnorm/l (it is the
same bf16 value for every key), so its rounding is harmless; s*j enters
through the fp32 ACT bias exactly.
"""

import math
import numpy as np
import ml_dtypes

import concourse.bass as bass
import concourse.mybir as mybir
import concourse.tile as tile
from concourse import bacc
from concourse.bass_utils import run_bass_kernel_spmd

f32 = mybir.dt.float32
f32r = mybir.dt.float32r
bf16 = mybir.dt.bfloat16
EXP = mybir.ActivationFunctionType.Exp

B, T, C = 2, 2048, 1024
H, HKV, HD = 16, 4, 64
G = H // HKV              # 4 query heads per core
GH = G * HD               # 256
QKV = GH + 2 * HD         # 384 projection cols per core
SCALE = 1.0 / math.sqrt(HD)
SHIFT = 4.0
NKT = T // 128            # 16 key blocks of 128
BAND = 1                  # extra key blocks attended before each 512-chunk

_CACHED_NC = None


def _build_nc(reps=1):
    nc = bacc.Bacc("TRN2", target_bir_lowering=False, debug=False)

    xT = nc.dram_tensor("xT", [C, T], bf16, kind="ExternalInput")
    wqkv = nc.dram_tensor("wqkv", [C, QKV], bf16, kind="ExternalInput")
    wo = nc.dram_tensor("wo", [GH, C], bf16, kind="ExternalInput")
    aux = nc.dram_tensor("aux", [4, T], bf16, kind="ExternalInput")
    sjcol = nc.dram_tensor("sjcol", [128, NKT], f32, kind="ExternalInput")
    y = nc.dram_tensor("y", [T, C], bf16, kind="ExternalOutput")

    with tile.TileContext(nc) as tc:
        for r in range(reps):
            _emit(nc, tc, xT, wqkv, wo, aux, sjcol, y, sfx=f"_{r}" if r else "")

    nc.finalize()
    return nc


def _emit(nc, tc, xT, wqkv, wo, aux, sjcol, y, sfx=""):
    import contextlib
    ctx = contextlib.ExitStack()
    with ctx:
        const = ctx.enter_context(tc.tile_pool(name="const" + sfx, bufs=1))
        xpool = ctx.enter_context(tc.tile_pool(name="xpool" + sfx, bufs=24))
        ptpool = ctx.enter_context(tc.tile_pool(name="ptpool" + sfx, bufs=8))
        vtpool = ctx.enter_context(tc.tile_pool(name="vtpool" + sfx, bufs=2))
        ypool = ctx.enter_context(tc.tile_pool(name="ypool" + sfx, bufs=4))
        lpool = ctx.enter_context(tc.tile_pool(name="lpool" + sfx, bufs=2))
        # PSUM: 4 banks of S^T tiles + 4 banks shared by osum/pb/yp/rp
        spool = ctx.enter_context(tc.tile_pool(name="spool" + sfx, bufs=4, space="PSUM"))
        pssm = ctx.enter_context(tc.tile_pool(name="pssm" + sfx, bufs=4, space="PSUM"))

        # ---- constants / persistent tensors ----
        wqkv_sb = const.tile([128, C // 128, QKV], bf16, name="wqkv_sb")
        wqkv_r = wqkv.rearrange("(o p) m -> p o m", p=128)
        for c8 in range(8):
            nc.sync.dma_start(wqkv_sb[:, c8, :], wqkv_r[:, c8, :])
        wo_sb = const.tile([128, GH // 128, C], bf16, name="wo_sb")
        nc.sync.dma_start(wo_sb, wo.rearrange("(o p) n -> p o n", p=128))
        sj_sb = const.tile([128, NKT], f32, name="sj_sb")
        nc.sync.dma_start(sj_sb, sjcol[:, :])

        # 65 = 64 k/q features + one augmentation row: kaug row 64 is all
        # ones, qaug row 64 is -(s*i + SHIFT), so their product applies the
        # per-query stabilizing shift inside the S^T matmul.
        KA = 65
        kaug = const.tile([KA, T], bf16, name="kaug")
        nc.sync.dma_start(kaug[64:65, :], aux[0:1, :])   # ones
        qaug = []
        for h in range(G):
            qh = const.tile([KA, T], bf16, name=f"qaug{h}")
            nc.sync.dma_start(qh[64:65, :], aux[2:3, :])  # negm
            qaug.append(qh)

        # PV lhsT layout [ones | zeros*63 | v]: osum row 0 = l (custom-DVE
        # reciprocal requires base partition 0), rows 64..128 = att (PSUM
        # multi-partition reads must start at 0 or 64).
        VW = 64 + HD
        v_sb = const.tile([128, NKT, VW], bf16, name="v_sb")
        nc.gpsimd.memset(v_sb[:, :, :], 0.0)
        for kt in range(NKT):
            nc.gpsimd.memset(v_sb[:, kt, 0:1], 1.0)

        att = [const.tile([128, T], bf16, name=f"att{c}") for c in range(2)]
        # 1/l values, one head per 32-aligned partition row (32*h), zeros
        # elsewhere; separate tensors per att chunk to avoid WAR serialization
        lrows = []
        for c in range(2):
            lr = const.tile([128, T], f32r, name=f"lrows{c}")
            nc.gpsimd.memset(lr.bitcast(f32), 0.0)
            lrows.append(lr)

        ident_f = const.tile([64, 64], f32, name="ident_f")
        nc.gpsimd.memset(ident_f, 0.0)
        nc.gpsimd.affine_select(
            out=ident_f, in_=ident_f, compare_op=mybir.AluOpType.not_equal,
            fill=1.0, base=0, pattern=[[-1, 64]], channel_multiplier=1)
        ident = const.tile([64, 64], bf16, name="ident")
        nc.vector.tensor_copy(ident, ident_f)

        # 0/1 head-selection matrices for the 1/l broadcast matmul:
        # esel[c][32h, p] = 1 iff head h owns partition p of att chunk c
        esel = []
        for c in range(2):
            e = const.tile([128, 128], f32r, name=f"esel{c}")
            nc.gpsimd.memset(e.bitcast(f32), 0.0)
            nc.gpsimd.memset(e[64 * c:64 * c + 1, 0:64].bitcast(f32), 1.0)
            nc.gpsimd.memset(e[64 * c + 32:64 * c + 33, 64:128].bitcast(f32), 1.0)
            esel.append(e)

        # causal mask on the 128-col diagonal window of a [128,512] P^T tile:
        # keep iff n - p - 128*r >= 0; columns >= 128*(r+1) are entirely
        # below the diagonal, columns < 128*r were never written/read.
        def causal_mask(pt_half, r):
            off = 128 * r
            nc.gpsimd.affine_select(
                out=pt_half[:, off:off + 128], in_=pt_half[:, off:off + 128],
                compare_op=mybir.AluOpType.is_ge, fill=0.0,
                base=0, pattern=[[1, 128]], channel_multiplier=-1)

        # ---- phase B: QKV^T projection for one 1024-column span ----
        _xts_cache = {}

        def emit_b_loads(tc2):
            tcol = tc2 * 1024
            xts = [[None] * 8 for _ in range(2)]
            for nn in range(2):
                for c8 in range(8):
                    xt = xpool.tile([128, 512], bf16, name=f"xt{tc2}_{nn}_{c8}", tag="xt")
                    nc.sync.dma_start(
                        xt, xT[c8 * 128:(c8 + 1) * 128,
                               tcol + nn * 512:tcol + (nn + 1) * 512])
                    xts[nn][c8] = xt
            _xts_cache[tc2] = xts

        def emit_b(tc2, mts=(2, 0, 1)):
            tcol = tc2 * 1024
            if tc2 not in _xts_cache:
                emit_b_loads(tc2)
            xts = _xts_cache[tc2]
            for mt in mts:
                for nn in range(2):
                    pcol = tcol + nn * 512
                    pb = pssm.tile([128, 512], f32, name=f"pqkv{tc2}_{mt}_{nn}", tag="osum")
                    for c8 in range(8):
                        nc.tensor.matmul(
                            pb,
                            lhsT=wqkv_sb[:, c8, mt * 128:(mt + 1) * 128],
                            rhs=xts[nn][c8],
                            start=(c8 == 0), stop=(c8 == 7))
                    if mt < 2:
                        nc.vector.tensor_copy(qaug[2 * mt][0:64, pcol:pcol + 512], pb[0:64, :])
                        nc.vector.tensor_copy(qaug[2 * mt + 1][0:64, pcol:pcol + 512], pb[64:128, :])
                    else:
                        nc.vector.tensor_copy(kaug[0:64, pcol:pcol + 512], pb[0:64, :])
                        vt = vtpool.tile([64, 512], bf16, name=f"vt{tc2}_{nn}", tag="vt")
                        nc.vector.tensor_copy(vt, pb[64:128, :])
                        for i in range(4):
                            pt_ps = pssm.tile([128, 64], bf16,
                                              name=f"ptr{tc2}_{nn}_{i}", tag="osum")
                            nc.tensor.transpose(pt_ps, vt[:, i * 128:(i + 1) * 128], ident)
                            nc.vector.tensor_copy(
                                v_sb[:, tc2 * 8 + nn * 4 + i, 64:64 + HD], pt_ps)

        # ---- phase D: output projection for a set of 128-query tiles ----
        def emit_d(qts):
            for qt in qts:
                ysb = ypool.tile([128, C], bf16, name=f"ysb{qt}", tag="ysb")
                for n2 in range(2):
                    yp = pssm.tile([128, 512], f32, name=f"yp{qt}_{n2}", tag="osum")
                    for c2 in range(2):
                        nc.tensor.matmul(yp,
                                         lhsT=att[c2][:, qt * 128:(qt + 1) * 128],
                                         rhs=wo_sb[:, c2, n2 * 512:(n2 + 1) * 512],
                                         start=(c2 == 0), stop=(c2 == 1))
                    if n2 == 0:
                        nc.vector.tensor_copy(ysb[:, n2 * 512:(n2 + 1) * 512], yp)
                    else:
                        nc.scalar.copy(ysb[:, n2 * 512:(n2 + 1) * 512], yp)
                nc.sync.dma_start(y[qt * 128:(qt + 1) * 128, :], ysb)

        # ---- phase C: banded attention for one 1024-query half ----
        # 256-query chunks: chunk qc attends key blocks [2qc-1, 2qc+1] (min
        # key distance 128 at chunk start, same band as validated).  Heads
        # are processed in pairs sharing one [128,2,256] S^T/P^T tile so a
        # single ACT instruction serves both heads of an att chunk.
        def emit_c(half, extra=None):
            slot = 0
            for hp in range(2):
                h0, h1 = 2 * hp, 2 * hp + 1
                for qcp in (2 * half, 2 * half + 1):
                    if extra is not None:
                        extra(slot)
                        slot += 1
                    osums = {}
                    for h in (h0, h1):
                        osums[h] = pssm.tile([128, 512], f32,
                                             name=f"os{half}_{hp}_{qcp}_{h}", tag="osum")
                    for qci, qc in enumerate((2 * qcp, 2 * qcp + 1)):
                        kts = [kt for kt in (2 * qc - 1, 2 * qc, 2 * qc + 1) if kt >= 0]
                        for kt in kts:
                            r = kt - 2 * qc
                            off = 128 * r if r > 0 else 0
                            sp2 = spool.tile([128, 2, 256], f32,
                                             name=f"sp{half}_{hp}_{qc}_{kt}", tag="sp")
                            pt2 = ptpool.tile([128, 2, 256], bf16,
                                              name=f"pt{half}_{hp}_{qc}_{kt}", tag="pt")
                            for hh, h in enumerate((h0, h1)):
                                nc.tensor.matmul(
                                    sp2[:, hh, off:256],
                                    lhsT=kaug[:, kt * 128:(kt + 1) * 128],
                                    rhs=qaug[h][:, qc * 256 + off:(qc + 1) * 256],
                                    start=True, stop=True)
                            nc.scalar.activation(pt2[:, :, off:256], sp2[:, :, off:256],
                                                 EXP, bias=sj_sb[:, kt:kt + 1])
                            for hh, h in enumerate((h0, h1)):
                                if r >= 0:
                                    nc.gpsimd.affine_select(
                                        out=pt2[:, hh, off:off + 128],
                                        in_=pt2[:, hh, off:off + 128],
                                        compare_op=mybir.AluOpType.is_ge, fill=0.0,
                                        base=0, pattern=[[1, 128]],
                                        channel_multiplier=-1)
                                nc.tensor.matmul(
                                    osums[h][:, qci * 256 + off:(qci + 1) * 256],
                                    lhsT=v_sb[:, kt, :], rhs=pt2[:, hh, off:256],
                                    start=(kt == kts[0]), stop=(kt == kts[-1]))
                    # evacuate + normalize (att chunk c2 == hp)
                    for h in (h0, h1):
                        halfrow = (h % 2) * 64
                        nc.vector.tensor_copy(
                            att[hp][halfrow:halfrow + 64, qcp * 512:(qcp + 1) * 512],
                            osums[h][64:64 + HD, :])
                        ls = lpool.tile([128, 512], f32r, name=f"ls{half}_{qcp}_{h}", tag="ls")
                        lsf = ls.bitcast(f32)
                        nc.vector.reciprocal_approx_fast(lsf[0:1, :], osums[h][0:1, :])
                        nc.sync.dma_start(
                            lrows[hp][32 * h:32 * h + 1, qcp * 512:(qcp + 1) * 512],
                            ls[0:1, :])
                    rp = pssm.tile([128, 512], f32, name=f"rp{half}_{hp}_{qcp}", tag="osum")
                    nc.tensor.matmul(rp, lhsT=esel[hp],
                                     rhs=lrows[hp][:, qcp * 512:(qcp + 1) * 512],
                                     start=True, stop=True)
                    nc.vector.tensor_tensor(att[hp][:, qcp * 512:(qcp + 1) * 512],
                                            att[hp][:, qcp * 512:(qcp + 1) * 512], rp,
                                            mybir.AluOpType.mult)

        emit_b(0)
        emit_b_loads(1)
        # interleave the second B span's projections into the first query
        # half's attention (C(0) only needs keys 0..1023, i.e. B span 0)
        _b1 = {0: (2,), 1: (0,), 2: (1,)}

        def _c0_extra(slot):
            if slot in _b1:
                emit_b(1, mts=_b1[slot])
        emit_c(0, extra=_c0_extra)
        # interleave the first query-half's output projection into the second
        # half's attention so PE fills ACT-bound gaps
        emit_c(1, extra=lambda slot: emit_d([2 * slot, 2 * slot + 1]))
        emit_d(range(8, 16))


def _alibi_slopes(n_heads):
    start = 2.0 ** (-(2.0 ** (-(math.log2(n_heads) - 3))))
    return np.array([start * (start ** i) for i in range(n_heads)], dtype=np.float32)


def kernel(x, Wq, Wk, Wv, Wo):
    global _CACHED_NC
    if _CACHED_NC is None:
        _CACHED_NC = _build_nc()
    nc = _CACHED_NC

    x = np.asarray(x, dtype=np.float32)
    Wq = np.asarray(Wq, dtype=np.float32)
    Wk = np.asarray(Wk, dtype=np.float32)
    Wv = np.asarray(Wv, dtype=np.float32)
    Wo = np.asarray(Wo, dtype=np.float32)

    slopes = _alibi_slopes(H)[:HKV]
    ar = np.arange(T, dtype=np.float32)
    bf = ml_dtypes.bfloat16

    in_maps = []
    for b in range(B):
        xT_b = np.ascontiguousarray(x[b].T.astype(bf))
        for g in range(HKV):
            s = float(slopes[g])
            wq_g = Wq[:, g * GH:(g + 1) * GH] * SCALE
            wk_g = Wk[:, g * HD:(g + 1) * HD]
            wv_g = Wv[:, g * HD:(g + 1) * HD]
            wqkv = np.ascontiguousarray(
                np.concatenate([wq_g, wk_g, wv_g], axis=1).astype(bf))
            wo_g = np.ascontiguousarray(Wo[g * GH:(g + 1) * GH, :].astype(bf))
            negm = -(s * ar + SHIFT)
            aux = np.ascontiguousarray(
                np.stack([np.ones(T, np.float32), np.zeros(T, np.float32),
                          negm, np.ones(T, np.float32)]).astype(bf))
            sjcol = np.ascontiguousarray((s * ar).reshape(NKT, 128).T)
            in_maps.append({
                "xT": xT_b, "wqkv": wqkv, "wo": wo_g,
                "aux": aux, "sjcol": sjcol,
            })

    global _last_in_maps
    _last_in_maps = in_maps
    res = run_bass_kernel_spmd(nc, in_maps, list(range(B * HKV)))
    out = np.zeros((B, T, C), dtype=np.float32)
    for b in range(B):
        for g in range(HKV):
            out[b] += res.results[b * HKV + g]["y"].astype(np.float32)
    return out
